# revision 15
# baseline (speedup 1.0000x reference)
"""Trainium2 Bass kernel for nn_EquivariantProductBasisBlock (v3).

Math: per (n,c) with x = node_feats[n,c,:] in R^9, one-hot node_attrs:
  f[n,c,dt] = sum_k w3[n,k,c] * <U3sym[dt,:,k], mono3(x)>
            + sum_k w2[n,k,c] * <U2sym[dt,:,k], mono2(x)>
            + sum_k w1[n,k,c] * <U1[dt,:,k], x>
  out = concat_dt(f @ Wlin) / sqrt(C) + sc

The axon dispatch is transfer/overhead-bound (~80ms fixed + HLO-size-
dependent compile-RPC + ~70MB/s data), so:
  - monomials are built ON DEVICE from x alone (fp16, ~0.6MB/core) via 0/1
    gather matmuls on the PE,
  - all coefficient matrices ride in the NEFF via inline_tensor,
  - per-slot element weights are expanded on device into WEcol/WE1col once
    per dispatch (one broadcast copy per element run), making every block
    identical so the whole body sits in one For_i hardware loop — the BIR
    stays ~100 instructions instead of ~1400, which keeps the per-call
    lower+compile RPC small,
  - output f is fp16.

Per 512-column block (4 node-slots x 128 channels, c-fastest), with
xb = XT[:, off:off+512] ([9, 512] fp16):
  m2ab_ps[109]  = S2ab.T @ xb        (m2 left ops | pad | m2 right ops)
  g4u_ps[76]    = SG4U.T @ xb        (m3-tail x-gather | pad | U1.T x)
  g2_ps[128]    = SG2.T  @ xb        (m3-head x-gather)
  m2[45]        = m2ab_ps[64:109] * copy(m2ab_ps[0:45])
  m3t1[128]     = (SL1.T @ m2) * copy(g2_ps)
  m3t2[37]      = (SL2.T @ m2) * copy(g4u_ps[0:37])
  G_ps[124]     = CFa.T @ m3t1 + CF2.T @ m2 + CF3t.T @ m3t2
  t1 = G_ps * WEcol[:, off:off+512];  t1u = g4u_ps[64:76] * WE1col[:, ...]
  f[4]          = R1.T @ t1 + R2.T @ t1u
Nodes are dealt to cores round-robin per element class so the slot->element
map is identical on all 8 cores (SPMD-uniform). Host: final equivariant
Linear + sc, inverse permutation.
"""
import sys
import numpy as np

sys.path.insert(0, "/opt/trn_rl_repo")

N, C, I, E = 2048, 128, 9, 10
K3, K2, K1 = 23, 8, 3
NCORES = 8
FB = 512                  # free cols per block
SLOTS_PER_BLK = FB // C   # 4 node-slots per block

TRI3 = [(a, b, c) for a in range(I) for b in range(a, I) for c in range(b, I)]
TRI2 = [(a, b) for a in range(I) for b in range(a, I)]
M2IDX = {ab: r for r, ab in enumerate(TRI2)}
NM3, NM2 = len(TRI3), len(TRI2)           # 165, 45
NC3, NC2, NC1 = 4 * K3, 4 * K2, 4 * K1    # 92, 32, 12
NCOL = NC3 + NC2                          # 124
MAR = 128                                 # m3 head rows
NT = NM3 - MAR                            # 37 m3 tail rows
DT_LIST = [(0, 0), (1, 0), (1, 1), (1, 2)]
UNROLL = 1                                # hw-loop unroll factor

_compiled = {}


def _build_consts(inputs):
    """Coefficient / selection matrices derived from the U/W input tensors."""
    U3s = [np.asarray(inputs["U3_0"]), np.asarray(inputs["U3_1"])]
    U2s = [np.asarray(inputs["U2_0"]), np.asarray(inputs["U2_1"])]
    U1s = [np.asarray(inputs["U1_0"]), np.asarray(inputs["U1_1"])]
    W3s = [np.asarray(inputs["W3_0"]), np.asarray(inputs["W3_1"])]
    W2s = [np.asarray(inputs["W2_0"]), np.asarray(inputs["W2_1"])]
    W1s = [np.asarray(inputs["W1_0"]), np.asarray(inputs["W1_1"])]

    CF3 = np.zeros((NM3, NCOL), np.float64)
    CF2 = np.zeros((NM2, NCOL), np.float64)
    tri3_idx = {m: r for r, m in enumerate(TRI3)}
    for di, (s, d) in enumerate(DT_LIST):
        u3 = np.zeros((NM3, K3), np.float64)
        u2 = np.zeros((NM2, K2), np.float64)
        U3 = np.asarray(U3s[s], np.float64)
        U2 = np.asarray(U2s[s], np.float64)
        for p in range(I):
            for q in range(I):
                u2[M2IDX[tuple(sorted((p, q)))]] += U2[d, p, q, :]
                for i in range(I):
                    u3[tri3_idx[tuple(sorted((p, q, i)))]] += U3[d, p, q, i, :]
        CF3[:, di * K3:(di + 1) * K3] = u3
        CF2[:, NC3 + di * K2:NC3 + (di + 1) * K2] = u2

    S1u = np.zeros((I, NC1), np.float32)
    for di, (s, d) in enumerate(DT_LIST):
        S1u[:, di * K1:(di + 1) * K1] = U1s[s][d, :, :]

    R1 = np.zeros((NCOL, 4), np.float16)
    R2 = np.zeros((NC1, 4), np.float16)
    # weight tables deduped: dt blocks with the same source irrep (s) share
    # rows, so ship unique rows + 0/1 replication matrices (expanded on device)
    WE32u = np.zeros((2 * (K3 + K2), E, C), np.float32)   # [62, E, C]
    WE1u = np.zeros((2 * K1, E, C), np.float32)           # [6, E, C]
    for s in range(2):
        WE32u[s * K3:(s + 1) * K3] = W3s[s].transpose(1, 0, 2)
        WE32u[2 * K3 + s * K2:2 * K3 + (s + 1) * K2] = W2s[s].transpose(1, 0, 2)
        WE1u[s * K1:(s + 1) * K1] = W1s[s].transpose(1, 0, 2)
    REP = np.zeros((2 * (K3 + K2), NCOL), np.float16)     # [62, 124]
    REP1 = np.zeros((2 * K1, NC1), np.float16)            # [6, 12]
    for di, (s, d) in enumerate(DT_LIST):
        R1[di * K3:(di + 1) * K3, di] = 1.0
        R1[NC3 + di * K2:NC3 + (di + 1) * K2, di] = 1.0
        R2[di * K1:(di + 1) * K1, di] = 1.0
        for k in range(K3):
            REP[s * K3 + k, di * K3 + k] = 1.0
        for k in range(K2):
            REP[2 * K3 + s * K2 + k, NC3 + di * K2 + k] = 1.0
        for k in range(K1):
            REP1[s * K1 + k, di * K1 + k] = 1.0

    # stacked gather matrices (pad sections keep psum reads at offsets 0/64)
    S2ab = np.zeros((I, 64 + NM2), np.float16)        # [9, 109]
    for j, (a, b) in enumerate(TRI2):
        S2ab[a, j] = 1.0
        S2ab[b, 64 + j] = 1.0

    SG2 = np.zeros((I, MAR), np.float16)              # [9, 128] x_c for m3 head
    SL1 = np.zeros((NM2, MAR), np.float16)            # [45, 128] m2 for m3 head
    for r in range(MAR):
        a, b, c = TRI3[r]
        SG2[c, r] = 1.0
        SL1[M2IDX[(a, b)], r] = 1.0

    SG4U = np.zeros((I, 64 + NC1), np.float16)        # [9, 76] x_c tail | pad | U1
    SL2 = np.zeros((NM2, NT), np.float16)             # [45, 37] m2 for m3 tail
    for t in range(NT):
        a, b, c = TRI3[MAR + t]
        SG4U[c, t] = 1.0
        SL2[M2IDX[(a, b)], t] = 1.0
    SG4U[:, 64:] = S1u.astype(np.float16)

    return {
        "S2ab": S2ab, "SG2": SG2, "SG4U": SG4U, "SL1": SL1, "SL2": SL2,
        "CFa": CF3[:MAR].astype(np.float16),
        "CF2m": CF2.astype(np.float16),
        "CF3t": CF3[MAR:].astype(np.float16),
        "R1": R1, "R2": R2, "REP": REP, "REP1": REP1,
        "WE32u": WE32u.reshape(2 * (K3 + K2), E * C).astype(np.float16),
        "WE1u": WE1u.reshape(2 * K1, E * C).astype(np.float16),
    }


def _build_nc(runs, nblk, consts):
    """Bass program; runs = ((elem, slot0, slot1), ...) same on all cores."""
    from concourse import bass, bacc, tile, mybir
    from concourse.bass import ds

    f32 = mybir.dt.float32
    f16 = mybir.dt.float16
    u8 = mybir.dt.uint8
    u16 = mybir.dt.uint16
    SOP = mybir.AluOpType
    FT = nblk * FB
    CB = 3 * FB // 2   # packed bytes per block: H(FB) | nibble-pairs(FB/2)

    nc = bacc.Bacc(None, target_bir_lowering=False, debug=False)
    xp_d = nc.declare_dram_parameter("XP", [I, nblk * CB], u8, isOutput=False)
    cd = {k: nc.inline_tensor(np.ascontiguousarray(v), name=k)
          for k, v in consts.items()}
    f_d = nc.declare_dram_parameter("f", [4, FT], f16, isOutput=True)

    with tile.TileContext(nc) as tc:
        with (
            tc.tile_pool(name="const", bufs=1) as cpool,
            tc.tile_pool(name="work", bufs=2) as wpool,
            tc.tile_pool(name="psum", bufs=1, space=bass.MemorySpace.PSUM) as pp,
        ):
            ct = {}
            for k, v in consts.items():
                ct[k] = cpool.tile(list(v.shape), f16, tag=k, name=k)
                nc.sync.dma_start(out=ct[k][:], in_=cd[k][:])

            # replicate unique weight rows to full dt-resolved tables (exact
            # 0/1 gather matmuls), then expand per-element weights to
            # per-column (slot-major, c-fastest)
            EC = E * C
            we32 = cpool.tile([NCOL, EC], f16, tag="we32f", name="we32f")
            we1 = cpool.tile([NC1, EC], f16, tag="we1f", name="we1f")
            for lo in range(0, EC, FB):
                w = min(FB, EC - lo)
                ps = pp.tile([NCOL, FB], f32, tag="g", name="wexp_ps")
                nc.tensor.matmul(ps[:, :w], ct["REP"][:],
                                 ct["WE32u"][:, lo:lo + w],
                                 start=True, stop=True)
                nc.scalar.copy(we32[:, lo:lo + w], ps[:, :w])
                ps1 = pp.tile([NC1, FB], f32, tag="g4u", name="wexp1_ps")
                nc.tensor.matmul(ps1[:, :w], ct["REP1"][:],
                                 ct["WE1u"][:, lo:lo + w],
                                 start=True, stop=True)
                nc.scalar.copy(we1[:, lo:lo + w], ps1[:, :w])

            wecol = cpool.tile([NCOL, FT], f16, tag="wecol", name="wecol")
            we1col = cpool.tile([NC1, FT], f16, tag="we1col", name="we1col")
            for (e, s0, s1) in runs:
                ns = s1 - s0
                nc.vector.tensor_copy(
                    wecol[:, s0 * C:s1 * C].rearrange("p (n c) -> p n c", n=ns),
                    we32[:, e * C:(e + 1) * C]
                    .unsqueeze(1).broadcast_to([NCOL, ns, C]))
                nc.vector.tensor_copy(
                    we1col[:, s0 * C:s1 * C].rearrange("p (n c) -> p n c", n=ns),
                    we1[:, e * C:(e + 1) * C]
                    .unsqueeze(1).broadcast_to([NC1, ns, C]))

            def body(bi):
                off = bi * FB
                # 12-bit x unpack: fp16 bits = H<<8 | nibble<<4 (low 4 zero)
                xraw = wpool.tile([I, CB], u8, tag="xraw")
                nc.sync.dma_start(out=xraw[:], in_=xp_d[:, ds(bi * CB, CB)])
                h16 = wpool.tile([I, FB], u16, tag="h16")
                nc.vector.tensor_copy(h16[:], xraw[:, 0:FB])
                n16 = wpool.tile([I, FB // 2], u16, tag="n16")
                nc.vector.tensor_copy(n16[:], xraw[:, FB:CB])
                xbits = wpool.tile([I, FB], u16, tag="xbits")
                nc.vector.tensor_scalar(xbits[:], h16[:], 8, None,
                                        op0=SOP.logical_shift_left)
                nlo = wpool.tile([I, FB // 2], u16, tag="nlo")
                nc.vector.tensor_scalar(nlo[:], n16[:], 15, 4,
                                        op0=SOP.bitwise_and,
                                        op1=SOP.logical_shift_left)
                nhi = wpool.tile([I, FB // 2], u16, tag="nhi")
                nc.vector.tensor_scalar(nhi[:], n16[:], 240, None,
                                        op0=SOP.bitwise_and)
                xpair = xbits[:].rearrange("p (k two) -> p k two", two=2)
                nc.vector.tensor_tensor(out=xpair[:, :, 0:1].squeeze(2),
                                        in0=xpair[:, :, 0:1].squeeze(2),
                                        in1=nlo[:], op=SOP.bitwise_or)
                nc.vector.tensor_tensor(out=xpair[:, :, 1:2].squeeze(2),
                                        in0=xpair[:, :, 1:2].squeeze(2),
                                        in1=nhi[:], op=SOP.bitwise_or)
                xb = xbits[:].bitcast(f16)

                m2ab_ps = pp.tile([64 + NM2, FB], f32, tag="m2ab")
                nc.tensor.matmul(m2ab_ps[:], ct["S2ab"][:], xb,
                                 start=True, stop=True)
                g4u_ps = pp.tile([64 + NC1, FB], f32, tag="g4u")
                nc.tensor.matmul(g4u_ps[:], ct["SG4U"][:], xb,
                                 start=True, stop=True)
                g2_ps = pp.tile([MAR, FB], f32, tag="g2")
                nc.tensor.matmul(g2_ps[:], ct["SG2"][:], xb,
                                 start=True, stop=True)

                m2a = wpool.tile([NM2, FB], f16, tag="m2a")
                nc.scalar.copy(m2a[:], m2ab_ps[0:NM2])
                m2 = wpool.tile([NM2, FB], f16, tag="m2")
                nc.vector.tensor_mul(m2[:], m2ab_ps[64:64 + NM2], m2a[:])

                g1_ps = pp.tile([MAR, FB], f32, tag="g1")
                nc.tensor.matmul(g1_ps[:], ct["SL1"][:], m2[:],
                                 start=True, stop=True)
                g2s = wpool.tile([MAR, FB], f16, tag="g2s")
                nc.scalar.copy(g2s[:], g2_ps[:])
                m3t1 = wpool.tile([MAR, FB], f16, tag="m3t1")
                nc.vector.tensor_mul(m3t1[:], g1_ps[:], g2s[:])

                g3_ps = pp.tile([NT, FB], f32, tag="g3")
                nc.tensor.matmul(g3_ps[:], ct["SL2"][:], m2[:],
                                 start=True, stop=True)
                g4s = wpool.tile([NT, FB], f16, tag="g4s")
                nc.scalar.copy(g4s[:], g4u_ps[0:NT])
                m3t2 = wpool.tile([NT, FB], f16, tag="m3t2")
                nc.vector.tensor_mul(m3t2[:], g3_ps[:], g4s[:])

                g_ps = pp.tile([NCOL, FB], f32, tag="g")
                nc.tensor.matmul(g_ps[:], ct["CFa"][:], m3t1[:],
                                 start=True, stop=False)
                nc.tensor.matmul(g_ps[:], ct["CF2m"][:], m2[:],
                                 start=False, stop=False)
                nc.tensor.matmul(g_ps[:], ct["CF3t"][:], m3t2[:],
                                 start=False, stop=True)

                t1 = wpool.tile([NCOL, FB], f16, tag="t1")
                nc.vector.tensor_mul(t1[:], g_ps[:], wecol[:, ds(off, FB)])
                t1u = wpool.tile([NC1, FB], f16, tag="t1u")
                nc.vector.tensor_mul(t1u[:], g4u_ps[64:64 + NC1],
                                     we1col[:, ds(off, FB)])

                f_ps = pp.tile([4, FB], f32, tag="f", bufs=2)
                nc.tensor.matmul(f_ps[:], ct["R1"][:], t1[:],
                                 start=True, stop=False)
                nc.tensor.matmul(f_ps[:], ct["R2"][:], t1u[:],
                                 start=False, stop=True)

                fout = wpool.tile([4, FB], f16, tag="fout")
                nc.scalar.copy(fout[:], f_ps[:])
                nc.sync.dma_start(out=f_d[:, ds(off, FB)], in_=fout[:])

            tc.For_i_unrolled(0, nblk, 1, body, max_unroll=UNROLL)

    nc.compile()

    # snapshot inline-const metadata: the bass2jax lowering converts Const
    # allocations to ExternalInput in place on first compile; restore after
    # each dispatch so retraces see the original (cached) program.
    from concourse import mybir as _mybir
    snap = {}
    for alloc in nc.m.functions[0].allocations:
        if isinstance(alloc, _mybir.MemoryLocationSet) and alloc.kind == "Const":
            snap[alloc.memorylocations[0].name] = (alloc.ant_data, alloc.file)
    return nc, snap


def _restore_consts(nc, snap):
    from concourse import mybir as _mybir
    for alloc in nc.m.functions[0].allocations:
        if not isinstance(alloc, _mybir.MemoryLocationSet):
            continue
        name = alloc.memorylocations[0].name
        if name in snap and alloc.kind != "Const":
            alloc.kind = "Const"
            alloc.ant_data, alloc.file = snap[name]


def kernel(**inputs):
    from concourse.bass_utils import run_bass_kernel_spmd

    x = np.ascontiguousarray(np.asarray(inputs["node_feats"], np.float32))
    sc = np.asarray(inputs["sc"], np.float32)
    y = np.asarray(inputs["node_attrs"], np.float32)
    Wlin0 = np.asarray(inputs["Wlin0"], np.float32)
    Wlin1 = np.asarray(inputs["Wlin1"], np.float32)

    elem = np.argmax(y, axis=1)
    consts = _build_consts(inputs)

    # deal nodes: element e's nodes round-robin over cores; slot runs are
    # slot-granular (blocks may span elements — WEcol handles per-slot e)
    count = np.bincount(elem, minlength=E)
    spe = [int(np.ceil(cnt / NCORES)) if cnt else 0 for cnt in count]
    base_slot = np.zeros(E + 1, np.int64)
    for e in range(E):
        base_slot[e + 1] = base_slot[e] + spe[e]
    NSLOT_used = int(base_slot[E])
    NBLK = (NSLOT_used + SLOTS_PER_BLK - 1) // SLOTS_PER_BLK
    NSLOT = NBLK * SLOTS_PER_BLK
    FT = NBLK * FB

    runs = []
    for e in range(E):
        if spe[e]:
            runs.append((e, int(base_slot[e]), int(base_slot[e + 1])))
    if runs:   # extend final run over tail pad slots (x=0 there -> f=0)
        e, s0, _ = runs[-1]
        runs[-1] = (e, s0, NSLOT)
    runs = tuple(runs)

    order = np.argsort(elem, kind="stable")
    core_of = np.empty(N, np.int64)
    slot_of = np.empty(N, np.int64)
    pos = 0
    for e in range(E):
        idx = order[pos:pos + count[e]]
        pos += count[e]
        for j, n_ in enumerate(idx):
            core_of[n_] = j % NCORES
            slot_of[n_] = base_slot[e] + j // NCORES

    import hashlib
    ch = hashlib.sha1()
    for k in sorted(consts):
        ch.update(consts[k].tobytes())
    key = (runs, NBLK, ch.hexdigest())
    if key not in _compiled:
        _compiled[key] = _build_nc(runs, NBLK, consts)
    nc, snap = _compiled[key]

    in_maps = []
    CB = 3 * FB // 2
    for core in range(NCORES):
        xc = np.zeros((NSLOT, C, I), np.float32)
        mask = core_of == core
        xc[slot_of[mask]] = x[mask]
        xT16 = xc.transpose(2, 0, 1).reshape(I, FT).astype(np.float16)
        # 12-bit pack: round fp16 to 12 bits (rne), split hi byte + nibbles
        q = ((xT16.view(np.uint16).astype(np.uint32) + 8) & 0xFFF0
             ).astype(np.uint16)
        H = (q >> 8).astype(np.uint8).reshape(I, NBLK, FB)
        nib = ((q >> 4) & 0xF).astype(np.uint8)
        NB = (nib[:, 0::2] | (nib[:, 1::2] << 4)).reshape(I, NBLK, FB // 2)
        XP = np.ascontiguousarray(
            np.concatenate([H, NB], axis=2).reshape(I, NBLK * CB))
        in_maps.append({"XP": XP})

    res = run_bass_kernel_spmd(nc, in_maps, list(range(NCORES)))
    _restore_consts(nc, snap)
    globals()["LAST_RESULT"] = res
    import os
    nrep = int(os.environ.get("KERNEL_TIME_RUNS", "8"))
    if nrep:
        import time
        times = []
        for _ in range(nrep):
            t0 = time.perf_counter()
            run_bass_kernel_spmd(nc, in_maps, list(range(NCORES)))
            times.append(time.perf_counter() - t0)
            _restore_consts(nc, snap)
        globals()["LAST_TIMES"] = times
    fcores = [np.asarray(r["f"]) for r in res.results]

    f_ncd = np.empty((N, C, 4), np.float32)
    for core in range(NCORES):
        fc = fcores[core].astype(np.float32).reshape(4, NSLOT, C)
        mask = core_of == core
        f_ncd[mask] = fc[:, slot_of[mask], :].transpose(1, 2, 0)

    inv = np.float32(1.0 / np.sqrt(C))
    y0 = np.einsum("nud,uw->nwd", f_ncd[:, :, :1], Wlin0) * inv
    y1 = np.einsum("nud,uw->nwd", f_ncd[:, :, 1:], Wlin1) * inv
    out = np.concatenate([y0.reshape(N, -1), y1.reshape(N, -1)], axis=-1) + sc
    return out.astype(np.float32)
